# revision 42
# baseline (speedup 1.0000x reference)
"""Modulated 3x3 conv (StyleGAN2-style, groups=B) on 8 trn2 NeuronCores.

Sharding: data-parallel over (batch B=4) x (image half H/2), 8 shards.
Each core computes a full 64->64 channel 3x3 conv over a 256-row half of
one sample's 512x512 image. Style modulation (s = affine(w)+1), weight
demodulation (d = rsqrt(sum (s*W)^2 + eps)) and the weight modulation are
computed on-device per core; the host only slices/pads/relayouts inputs.

Conv strategy per core: output rows are processed in pairs. SBUF holds x
rows in an even/odd interleaved layout: partitions 0-63 = Cin of even
rows, 64-127 = Cin of odd rows, each row stored with 1 zero pad column on
each side (514 cols/row). One output row pair accumulates 6 matmuls
(K=128, M=128, N=512) into one PSUM bank: 2 x-row-pairs x 3 kw shifts,
with the (kh-selecting) weights packed into six 128x128 stationary tiles
(two quadrants of which are structurally zero -> 75% PE utilization).
Matmul operands use float32r (TF32), which streams 1 column/cycle vs
plain fp32's 4; data movement stays fp32. The demodulation scale d is
folded into the PSUM->SBUF copy as a per-partition scalar multiply.
"""

import numpy as np

import concourse.bacc as bacc
import concourse.mybir as mybir
import concourse.tile as tile
from concourse.bass_utils import run_bass_kernel_spmd

B, CIN, COUT, L, H, W = 4, 64, 64, 512, 512, 512
KH = KW = 3
N_CORES = 8
HALF = H // 2  # 256 output rows per core
R_IN = HALF + 2  # 258 input rows per core (1 halo/pad row each side)
NPAIR = HALF // 2  # 128 output row pairs per core
G = 8  # row pairs per output group
NOG = NPAIR // G  # output groups = 16
ROWB = W + 2  # per-row SBUF columns (1 zero pad each side)
EPS = 1e-8
F32 = mybir.dt.float32
MM_DT = mybir.dt.float32r  # TF32-rate matmuls
# x pair load groups: fine-grained at the start so compute ramps early
LG = 8  # pairs per x load tile
LOAD_GROUPS = [(0, 2), (2, 2), (4, 4)] + [(8 * i, 8) for i in range(1, 16)] + [(128, 1)]
XBUFS = 6
# 'swdge': one cast-DMA per group (gpsimd); 'staged': contiguous HWDGE load
# into fp32 staging + ACT copy (does the TF32 rounding) into the padded tile
LOAD_MODE = "swdge"

_CACHE = {}


def _build_nc(reps=1):
    nc = bacc.Bacc("TRN2", target_bir_lowering=False, debug=False)
    # xs layout: [parity, ci, row-pair, w]; xs[s, ci, q, :] = padded row 2q+s
    xs = nc.dram_tensor("xs", [2, CIN, R_IN // 2, W], F32, kind="ExternalInput")
    wvec = nc.dram_tensor("wvec", [L], F32, kind="ExternalInput")
    wgt_t = nc.dram_tensor("wgt_t", [CIN, COUT * 9], F32, kind="ExternalInput")
    lhsT6 = nc.dram_tensor("lhsT6", [6, 128, 128], F32, kind="ExternalInput")
    affw = nc.dram_tensor("affw", [CIN, L], F32, kind="ExternalInput")
    affb = nc.dram_tensor("affb", [CIN], F32, kind="ExternalInput")
    # out layout: [parity, co, row-pair, w]; out[s, co, q, :] = out row 2q+s
    out = nc.dram_tensor("out", [2, COUT, NPAIR, W], F32, kind="ExternalOutput")

    with tile.TileContext(nc) as tc:
        for _ in range(reps):
            _emit(tc, xs, wvec, wgt_t, lhsT6, affw, affb, out)
    nc.compile()
    return nc


def _emit(tc, xs, wvec, wgt_t, lhsT6, affw, affb, out):
    nc = tc.nc
    const = tc.alloc_tile_pool(name="const", bufs=1)
    pprep = tc.alloc_tile_pool(name="pprep", bufs=1, space="PSUM")
    xpool = tc.alloc_tile_pool(name="xg", bufs=XBUFS)
    opool = tc.alloc_tile_pool(name="og", bufs=2 if LOAD_MODE == "staged" else 3)
    spool = (
        tc.alloc_tile_pool(name="xstg", bufs=2) if LOAD_MODE == "staged" else None
    )
    pmain = tc.alloc_tile_pool(name="pmain", bufs=6, space="PSUM")

    # ---------------- style vector s (duplicated across partition halves) ----
    # s2[p] = s[p % 64] = (affw @ wvec + affb + 1)[p % 64]
    aff2 = const.tile([128, L], F32, tag="aff2")
    nc.sync.dma_start(aff2[0:64, :], affw[:, :])
    nc.sync.dma_start(aff2[64:128, :], affw[:, :])
    wrow = const.tile([1, L], F32, tag="wrow")
    nc.sync.dma_start(wrow[:], wvec.rearrange("(u l) -> u l", u=1))
    onesrow = const.tile([1, 128], F32, tag="onesrow")
    nc.vector.memset(onesrow[:], 1.0)
    wb_ps = pprep.tile([128, L], F32, tag="pp")
    nc.tensor.matmul(wb_ps[:], onesrow[:], wrow[:])  # broadcast w across partitions
    scr = const.tile([128, L], F32, tag="scr")
    s2_raw = const.tile([128, 1], F32, tag="s2_raw")
    nc.vector.tensor_mul(scr[:], aff2[:], wb_ps[:])
    nc.vector.reduce_sum(
        s2_raw[:], scr[:], axis=mybir.AxisListType.X
    )
    affb2 = const.tile([128, 1], F32, tag="affb2")
    for half in range(2):
        nc.sync.dma_start(
            affb2[half * 64 : (half + 1) * 64, :], affb.rearrange("(c u) -> c u", u=1)
        )
    affb2p1 = const.tile([128, 1], F32, tag="affb2p1")
    nc.vector.tensor_scalar_add(affb2p1[:], affb2[:], 1.0)
    s2 = const.tile([128, 1], F32, tag="s2")
    nc.scalar.activation(
        s2[:], s2_raw[:], mybir.ActivationFunctionType.Identity, bias=affb2p1[:]
    )

    # ---------------- demodulation scale d ----------------------------------
    # dsq[co] = sum_{ci,kh,kw} (s[ci] * wgt[co,ci,kh,kw])^2
    wT = const.tile([64, COUT * 9], F32, tag="wT")  # [ci, co*9 + kh*3 + kw]
    nc.sync.dma_start(wT[:], wgt_t[:, :])
    swT = const.tile([64, COUT * 9], F32, tag="swT")
    nc.vector.tensor_scalar_mul(swT[:], wT[:], s2[0:64, :])
    nc.vector.tensor_mul(swT[:], swT[:], swT[:])
    qsum = const.tile([64, COUT], F32, tag="qsum")
    nc.vector.reduce_sum(
        qsum[:].rearrange("p (c u) -> p c u", u=1),
        swT[:].rearrange("p (c t) -> p c t", t=9),
        axis=mybir.AxisListType.X,
    )
    ones64 = const.tile([64, 1], F32, tag="ones64")
    nc.vector.memset(ones64[:], 1.0)
    ps_d = pprep.tile([1, COUT], F32, tag="pp")
    nc.tensor.matmul(ps_d[:], ones64[:], qsum[:])
    drow = const.tile([1, 128], F32, tag="drow")
    nc.vector.tensor_scalar_add(drow[:, 0:64], ps_d[:], EPS)
    nc.vector.tensor_scalar_add(drow[:, 64:128], ps_d[:], EPS)
    nc.vector.reciprocal(drow[:], drow[:])
    nc.scalar.activation(drow[:], drow[:], mybir.ActivationFunctionType.Sqrt)
    one1 = const.tile([1, 1], F32, tag="one1")
    nc.vector.memset(one1[:], 1.0)
    ps_dcol = pprep.tile([128, 1], F32, tag="pp")
    nc.tensor.matmul(ps_dcol[:], drow[:], one1[:])
    dcol = const.tile([128, 1], F32, tag="dcol")
    nc.vector.tensor_copy(dcol[:], ps_dcol[:])

    # ---------------- six stationary conv weight tiles -----------------------
    # lhsT6 (host-layouted, unmodulated): k=kw for A tiles, k=3+kw for B.
    # rhs partition p = sigma*64+ci holds x row (2q+sigma); psum partition
    # m = tau*64+co holds out row (2p+tau).  A pairs with x-pair p, B with
    # x-pair p+1; zero quadrants A(s0,t1), B(s1,t0) are baked into lhsT6.
    wstg = const.tile([128, 6 * 128], F32, tag="wstg")
    nc.sync.dma_start(
        wstg[:].rearrange("p (k m) -> p k m", m=128),
        lhsT6.rearrange("k p m -> p k m"),
    )
    WTILES = []
    for k in range(6):
        t = const.tile([128, 128], MM_DT, tag=f"WT{k}")
        stg = wstg[:, k * 128 : (k + 1) * 128]
        nc.vector.tensor_scalar_mul(t[0:64, :], stg[0:64, :], s2[0:64, :])
        nc.vector.tensor_scalar_mul(t[64:128, :], stg[64:128, :], s2[64:128, :])
        WTILES.append(t)
    AT, BT = WTILES[0:3], WTILES[3:6]

    # ---------------- main loop ----------------------------------------------
    xsv = xs.rearrange("s ci q w -> (s ci) q w")  # [128, R_IN//2, W]
    outv = out.rearrange("s co q w -> (s co) q w")  # [128, NPAIR, W]
    xslot = {}  # pair index -> (tile, slot)
    li = [0]  # next load group to issue

    def load_group(gi):
        q0, n = LOAD_GROUPS[gi]
        t = xpool.tile([128, LG * ROWB], MM_DT, tag="xg")
        v = t[:].rearrange("p (j c) -> p j c", c=ROWB)
        nc.vector.memset(v[:, 0:n, 0:1].bitcast(F32), 0.0)
        nc.vector.memset(v[:, 0:n, 513:514].bitcast(F32), 0.0)
        if LOAD_MODE == "swdge":
            # single 128-partition SWDGE DMA; casts fp32 -> fp32r (TF32 round)
            nc.gpsimd.dma_start(v[:, 0:n, 1:513], xsv[:, q0 : q0 + n, :])
        else:
            # fully-contiguous HWDGE load; ACT copy rounds into padded layout
            stg = spool.tile([128, G * W], F32, tag="xstg")
            sv = stg[:].rearrange("p (j c) -> p j c", c=W)
            nc.sync.dma_start(sv[:, 0:n, :], xsv[:, q0 : q0 + n, :])
            nc.scalar.activation(
                v[:, 0:n, 1:513], sv[:, 0:n, :], mybir.ActivationFunctionType.Copy
            )
        for i in range(n):
            xslot[q0 + i] = (t, i)

    def ensure_loaded(pair_needed):
        while li[0] < len(LOAD_GROUPS) and LOAD_GROUPS[li[0]][0] <= pair_needed:
            load_group(li[0])
            li[0] += 1

    def compute_group(g):
        og = opool.tile([128, G * 512], F32, tag="og")
        for j in range(G):
            p = g * G + j
            ps = pmain.tile([128, 512], F32, tag="ps")
            k = 0
            for (t, sl), WTs in zip((xslot[p], xslot[p + 1]), (AT, BT)):
                base = sl * ROWB
                for kw in range(KW):
                    nc.tensor.matmul(
                        ps[:],
                        WTs[kw][:],
                        t[:, base + kw : base + kw + 512],
                        start=(k == 0),
                        stop=(k == 5),
                    )
                    k += 1
            nc.vector.tensor_scalar_mul(
                og[:, j * 512 : (j + 1) * 512], ps[:], dcol[:, 0:1]
            )
        ogv = og[:].rearrange("p (j w) -> p j w", w=512)
        half_g = G // 2
        nc.sync.dma_start(
            outv[:, G * g : G * g + half_g, :], ogv[:, 0:half_g, :]
        )
        nc.sync.dma_start(
            outv[:, G * g + half_g : G * (g + 1), :], ogv[:, half_g:G, :]
        )

    for g in range(NOG):
        ensure_loaded(G * (g + 1) + 16)
        compute_group(g)

    pools = [pmain, opool, xpool, pprep, const]
    if spool is not None:
        pools.insert(1, spool)
    for p in pools:
        p.release()


def _get_nc(reps=1):
    if reps not in _CACHE:
        _CACHE[reps] = _build_nc(reps)
    return _CACHE[reps]


def _host_weight_layouts(weight):
    """Unmodulated stationary tiles [6,128,128]: A[kw]=k, B[kw]=3+kw."""
    lhsT6 = np.zeros((6, 128, 128), dtype=np.float32)
    wt = np.ascontiguousarray(weight.transpose(1, 0, 2, 3))  # [ci, co, kh, kw]
    for kw in range(3):
        a, b = lhsT6[kw], lhsT6[3 + kw]
        a[0:64, 0:64] = wt[:, :, 0, kw]
        a[64:128, 0:64] = wt[:, :, 1, kw]
        a[64:128, 64:128] = wt[:, :, 0, kw]
        b[0:64, 0:64] = wt[:, :, 2, kw]
        b[0:64, 64:128] = wt[:, :, 1, kw]
        b[64:128, 64:128] = wt[:, :, 2, kw]
    wgt_t = np.ascontiguousarray(wt.reshape(CIN, COUT * 9))
    return wgt_t, lhsT6


def _shard_inputs(x, w, weight, affine_w, affine_b):
    """Build the 8 per-core input maps (host-side slicing + halo padding)."""
    wgt_t, lhsT6 = _host_weight_layouts(np.asarray(weight, dtype=np.float32))
    affw = np.ascontiguousarray(np.asarray(affine_w, dtype=np.float32))
    affb = np.ascontiguousarray(np.asarray(affine_b, dtype=np.float32))
    in_maps = []
    for core in range(N_CORES):
        b, half = divmod(core, 2)
        h0 = half * HALF
        xsh = np.zeros((CIN, R_IN, W), dtype=np.float32)
        lo, hi = h0 - 1, h0 + HALF + 1  # global rows [lo, hi)
        clo, chi = max(lo, 0), min(hi, H)
        xsh[:, clo - lo : chi - lo, :] = x[b, :, clo:chi, :]
        # parity-split layout: xs2[s, ci, q, :] = xsh[ci, 2q+s, :]
        xs2 = np.ascontiguousarray(
            xsh.reshape(CIN, R_IN // 2, 2, W).transpose(2, 0, 1, 3)
        )
        in_maps.append(
            {
                "xs": xs2,
                "wvec": np.ascontiguousarray(w[b]).astype(np.float32),
                "wgt_t": wgt_t,
                "lhsT6": lhsT6,
                "affw": affw,
                "affb": affb,
            }
        )
    return in_maps


def kernel(x, w, weight, affine_w, affine_b):
    x = np.asarray(x, dtype=np.float32)
    w = np.asarray(w, dtype=np.float32)

    nc = _get_nc()
    in_maps = _shard_inputs(x, w, weight, affine_w, affine_b)
    res = run_bass_kernel_spmd(nc, in_maps, list(range(N_CORES)))
    full = np.empty((B, COUT, H, W), dtype=np.float32)
    for core in range(N_CORES):
        b, half = divmod(core, 2)
        o2 = res.results[core]["out"]  # [2, COUT, NPAIR, W]
        full[b, :, half * HALF : (half + 1) * HALF, :] = (
            o2.transpose(1, 2, 0, 3).reshape(COUT, HALF, W)
        )
    return full
